# revision 33
# baseline (speedup 1.0000x reference)
"""Causal multi-head self-attention on 8 trn2 NeuronCores.

Sharding: core c = (batch, head_group): batch = c // 4, heads = [4*(c%4) .. 4*(c%4)+3].
Each core computes the QKV projection for its batch + 4 heads, causal attention,
and a row-parallel slice of the output projection; the host sums the 4 partial
outputs per batch element.

v2 design (vs v1 baseline at ~210us):
 - bf16 data path end to end: host pre-casts x/w to bf16, all SBUF operands and
   the DRAM output are bf16 (PSUM accumulation stays fp32).  Halves DMA bytes,
   LDWEIGHTS size and DVE copy time; PE rate is 1 cyc/row either way, and bf16
   lifts fp32r's moving-dim>=256 restriction so causal raggedness is exact.
 - attention inner loop is software-pipelined: the PV matmul for j-tile jt is
   emitted one iteration behind the score matmul, so exp (ACT) latency never
   stalls the in-order PE queue.
 - projection / output-projection work is queued as "filler" units and emitted
   between attention ops at ~1-group granularity to keep the PE continuously
   busy (TRN2 DVFS: the PE only reaches 2.4 GHz after ~3us without gaps).
 - scores are computed transposed, ST[j,i] = (k_j . q_i)/8, softmax denominator
   comes from a ones-column appended to V (M=65 PV matmul), denominator is
   broadcast across partitions with a K=1 fp32r matmul and inverted on DVE.
 - no max-subtraction in softmax: scores are ~N(0,1), exp is safe.
"""

import numpy as np
from contextlib import ExitStack
from ml_dtypes import bfloat16

import concourse.bass as bass
from concourse import bacc
import concourse.mybir as mybir
import concourse.tile as tile
from concourse.bass_utils import run_bass_kernel_spmd

B, T, D, H, HD = 2, 2048, 1024, 16, 64
NCORES = 8
HPC = 4  # heads per core

f32 = mybir.dt.float32
R = mybir.dt.float32r
BF = mybir.dt.bfloat16
Exp = mybir.ActivationFunctionType.Exp

LAST_RESULTS = None  # BassKernelResults of the most recent kernel() call


def build_bass(t=T):
    """Build the per-core Bass program (SPMD: same program, different data)."""
    assert t % 512 == 0
    nci = t // 512      # 512-wide i-chunks
    njt_tot = t // 128  # 128-wide j-tiles

    nc = bacc.Bacc("TRN2", target_bir_lowering=False)
    xt = nc.dram_tensor("xt", [D, t], BF, kind="ExternalInput")
    wqk = nc.dram_tensor("wqk", [D, 512], BF, kind="ExternalInput")
    wv = nc.dram_tensor("wv", [D, 256], BF, kind="ExternalInput")
    wo = nc.dram_tensor("wo", [128, 2, D], BF, kind="ExternalInput")
    outp = nc.dram_tensor("outp", [D, t], BF, kind="ExternalOutput")

    xt_r = xt.rearrange("(kt p) t -> p kt t", p=128)      # [128, 8, t]
    wqk_r = wqk.rearrange("(kt p) f -> p kt f", p=128)    # [128, 8, 512]
    wv_r = wv.rearrange("(kt p) f -> p kt f", p=128)      # [128, 8, 256]
    outp_r = outp.rearrange("(ot p) t -> p ot t", p=128)  # [128, 8, t]

    with ExitStack() as ctx:
        tc = ctx.enter_context(tile.TileContext(nc))
        persist = ctx.enter_context(tc.tile_pool(name="persist", bufs=1))
        exps = ctx.enter_context(tc.tile_pool(name="exps", bufs=4))
        otn_pool = ctx.enter_context(tc.tile_pool(name="otn", bufs=4))
        rcp_pool = ctx.enter_context(tc.tile_pool(name="rcp", bufs=4))
        osb_pool = ctx.enter_context(tc.tile_pool(name="osb", bufs=3))
        ppsum = ctx.enter_context(tc.tile_pool(name="ppsum", bufs=2, space="PSUM"))
        spsum = ctx.enter_context(tc.tile_pool(name="spsum", bufs=2, space="PSUM"))
        pvpsum = ctx.enter_context(tc.tile_pool(name="pvpsum", bufs=1, space="PSUM"))

        # --- input DMAs, priority order; spread across engine queues.
        # Each DMA trigger costs ~640ns on the issuing queue, so batch into
        # few descriptors; split the first-needed tensors so kt=0 lands fast.
        dmaq = [nc.sync, nc.gpsimd]
        qi = [0]

        def dma(out, in_):
            dmaq[qi[0] % len(dmaq)].dma_start(out=out, in_=in_)
            qi[0] += 1

        wqk_sb = persist.tile([128, 8, 512], BF, tag="wqk_sb", name="wqk_sb")
        xin_t = [persist.tile([128, 8, 512], BF, tag=f"xin{ci}", name=f"xin{ci}")
                 for ci in range(nci)]
        # stream the first proj's operands per-kt so the first matmul starts
        # as soon as kt=0 lands (128 KB), and kt slices keep pace with the
        # accumulation chain; everything else goes as bulk descriptors after.
        for kt in range(8):
            dma(wqk_sb[:, kt, :], wqk_r[:, kt, :])
            dma(xin_t[0][:, kt, :], xt_r[:, kt, 0:512])
        wv_sb = persist.tile([128, 8, 256], BF, tag="wv_sb", name="wv_sb")
        dma(wv_sb[:, 0:4, :], wv_r[:, 0:4, :])
        dma(wv_sb[:, 4:8, :], wv_r[:, 4:8, :])
        for ci in range(1, nci):
            dma(xin_t[ci][:, 0:4, :], xt_r[:, 0:4, ci * 512:(ci + 1) * 512])
            dma(xin_t[ci][:, 4:8, :], xt_r[:, 4:8, ci * 512:(ci + 1) * 512])
        wo_sb = persist.tile([128, 2, D], BF, tag="wo_sb", name="wo_sb")
        dma(wo_sb, wo[:])

        # v with a PREPENDED 64-wide ones BLOCK: the PV matmul (M=128, same
        # cycle cost as M=65 since cost = moving cols) then lands the softmax
        # denominator replicated on PSUM partitions 0..63, so the reciprocal
        # reads PSUM directly - no single-partition denominator copy and no
        # K=1 broadcast matmul in the drain chain.  (Ones must come FIRST:
        # reciprocal_approx_fast only works on partitions 0..63, its internal
        # constants live there.)
        v_sb = persist.tile([128, njt_tot, HPC, 2 * HD], BF, tag="v_sb", name="v_sb")
        nc.gpsimd.memset(v_sb[:, :, :, 0:HD], 1.0)

        # qk_sb[ft][ci]: ft 0=q pair0, 1=k pair0, 2=q pair1, 3=k pair1
        # each tile [128, 512]: partitions 0:64 head A dims, 64:128 head B dims
        qk_sb = [[persist.tile([128, 512], BF, tag=f"qk_{ft}_{ci}", name=f"qk_{ft}_{ci}")
                  for ci in range(nci)] for ft in range(4)]

        # --- filler units: each is an atomic closure w.r.t. ppsum ---
        def proj_qk_group(ci, ft):
            def emit():
                ps = ppsum.tile([128, 512], f32, tag="mm512", name="pp")
                for kt in range(8):
                    nc.tensor.matmul(
                        ps,
                        lhsT=wqk_sb[:, kt, ft * 128:(ft + 1) * 128],
                        rhs=xin_t[ci][:, kt, :],
                        start=(kt == 0), stop=(kt == 7),
                    )
                nc.vector.tensor_copy(out=qk_sb[ft][ci], in_=ps)
            return emit

        def proj_v_group(ci, it):
            def emit():
                ps = ppsum.tile([128, 512], f32, tag="mm512", name="pp")
                for kt in range(8):
                    nc.tensor.matmul(
                        ps[:, 0:256],
                        lhsT=xin_t[ci][:, kt, it * 128:(it + 1) * 128],
                        rhs=wv_sb[:, kt, :],
                        start=(kt == 0), stop=(kt == 7),
                    )
                jt = ci * 4 + it
                nc.vector.tensor_copy(
                    out=v_sb[:, jt, :, HD:2 * HD],
                    in_=ps[:, 0:256].rearrange("p (h d) -> p h d", h=HPC),
                )
            return emit

        def outproj_unit(ci, ot, otn_ci, tail=False):
            def emit():
                ps = ppsum.tile([128, 512], f32, tag="mm512", name="pp")
                for pair in range(2):
                    nc.tensor.matmul(
                        ps,
                        lhsT=wo_sb[:, pair, ot * 128:(ot + 1) * 128],
                        rhs=otn_ci[pair],
                        start=(pair == 0), stop=(pair == 1),
                    )
                osb = osb_pool.tile([128, 512], BF, tag="osb", name="osb")
                if tail and ot % 2 == 1:
                    # final drain: split copies DVE/ACT so neither engine's
                    # queue serializes the tail (ACT has no exp left by then)
                    nc.scalar.activation(out=osb, in_=ps,
                                         func=mybir.ActivationFunctionType.Copy)
                else:
                    nc.vector.tensor_copy(out=osb, in_=ps)
                nc.sync.dma_start(
                    out=outp_r[:, ot, ci * 512:(ci + 1) * 512], in_=osb
                )
            return emit

        fillers = []

        def fill(n):
            for _ in range(min(n, len(fillers))):
                fillers.pop(0)()

        def proj0_stream():
            # startup: kt-major, two psum accumulators at a time, so each
            # arriving (wqk, xin0) kt-slice immediately feeds 2x512 cols of
            # matmul instead of every ft-chain stalling on the last slice.
            for fts in ((0, 1), (2, 3)):
                ps2 = [ppsum.tile([128, 512], f32, tag="mm512", name="pp")
                       for _ in fts]
                for kt in range(8):
                    for i, ft in enumerate(fts):
                        nc.tensor.matmul(
                            ps2[i],
                            lhsT=wqk_sb[:, kt, ft * 128:(ft + 1) * 128],
                            rhs=xin_t[0][:, kt, :],
                            start=(kt == 0), stop=(kt == 7),
                        )
                for i, ft in enumerate(fts):
                    nc.vector.tensor_copy(out=qk_sb[ft][0], in_=ps2[i])
            for it in range(4):
                proj_v_group(0, it)()

        def proj_units(ci):
            return ([proj_qk_group(ci, ft) for ft in range(4)]
                    + [proj_v_group(ci, it) for it in range(4)])

        def make_pair(ci, pair, otn_ci):
            """Closures for one (chunk, head-pair) attention tile stream.

            j-tiles are processed DIAG-FIRST: the diagonal tiles' long
            exp->affine_select chain hides behind the previous pair's tail,
            and the pair then ends on off-diagonal tiles whose PE work is
            large enough to cover plain exp latency.
            """
            njt = 4 * (ci + 1)
            order = list(range(4 * ci, njt)) + list(range(0, 4 * ci))
            first, last = order[0], order[-1]
            qtile = qk_sb[2 * pair][ci]
            st = {}

            def se(jt):
                d = jt - 4 * ci
                ioff = max(0, d * 128)   # causal-valid i starts here
                ktile = qk_sb[2 * pair + 1][jt // 4]
                ksl = ktile[:, (jt % 4) * 128:(jt % 4 + 1) * 128]
                sp = spsum.tile([128, 2, 512], f32, tag="sp", name="sp")
                nc.tensor.matmul(
                    sp[:, 0, ioff:512], lhsT=ksl[0:64, :],
                    rhs=qtile[0:64, ioff:512],
                )
                nc.tensor.matmul(
                    sp[:, 1, ioff:512], lhsT=ksl[64:128, :],
                    rhs=qtile[64:128, ioff:512],
                )
                ex = exps.tile([128, 2, 512], BF, tag="ex", name="ex")
                # exp((k.q)/sqrt(64)); PSUM -> SBUF bf16, both heads one call
                nc.scalar.activation(
                    out=ex[:, :, ioff:512], in_=sp[:, :, ioff:512],
                    func=Exp, scale=0.125,
                )
                if d >= 0:
                    # zero the upper triangle of the diagonal 128-block,
                    # both heads in one strided call (hh dim contributes 0)
                    nc.gpsimd.affine_select(
                        out=ex[:, :, ioff:ioff + 128],
                        in_=ex[:, :, ioff:ioff + 128],
                        compare_op=mybir.AluOpType.is_ge,
                        fill=0.0, base=0, channel_multiplier=-1,
                        pattern=[[0, 2], [1, 128]],
                    )
                st[jt] = (ioff, ex)

            def pv1(jt, hh):
                if jt == first and hh == 0:
                    st['pv'] = pvpsum.tile([128, 2, 512], f32, tag="pv",
                                           name="pv")
                ioff, ex = st[jt]
                nc.tensor.matmul(
                    st['pv'][:, hh, ioff:512],
                    lhsT=v_sb[:, jt, 2 * pair + hh, :],
                    rhs=ex[:, hh, ioff:512],
                    start=(jt == first), stop=(jt == last),
                )

            def pv(jt):
                pv1(jt, 0)
                pv1(jt, 1)
                del st[jt]

            def rcp_stt(hh):
                # denominator sits replicated on pv partitions 0..63
                if hh == 0:
                    st['otn'] = otn_pool.tile([128, 512], BF, tag="otn",
                                              name="otn")
                rcp = rcp_pool.tile([HD, 512], f32, tag="rcp", name="rcp")
                nc.vector.reciprocal_approx_fast(out=rcp,
                                                 in_=st['pv'][0:HD, hh, :])
                nc.vector.scalar_tensor_tensor(
                    out=st['otn'][hh * HD:(hh + 1) * HD, :],
                    in0=st['pv'][HD:2 * HD, hh, :],
                    scalar=1.0, in1=rcp,
                    op0=mybir.AluOpType.mult, op1=mybir.AluOpType.mult,
                )
                if hh == 1:
                    otn_ci.append(st['otn'])

            return order, se, pv, pv1, rcp_stt

        # --- schedule ---
        # proj0 inline upfront (kt-major, streaming with the DMA); then one
        # linear walk over the 8 (chunk, pair) attention streams with:
        #  - PV lagging scores by 2 j-tiles (exp/mask latency never stalls PE)
        #  - the next pair's first two score/exp tiles emitted during this
        #    pair's tail, so the softmax pipeline never refills from empty
        #  - the drain split so DVE reciprocal overlaps the last PV matmuls
        #  - proj(ci+1) and outproj(ci-1) units paced as fillers between
        #    attention ops (keeps the PE DVFS clock at max)
        proj0_stream()
        otn_cis = [[] for _ in range(nci)]
        seq = []
        for ci in range(nci):
            for pair in range(2):
                seq.append((ci, pair) + make_pair(ci, pair, otn_cis[ci]))

        pace = {'due': 0.0}

        def pace_fill(weight):
            pace['due'] += weight
            while pace['due'] >= 1.0 and fillers:
                pace['due'] -= 1.0
                fill(1)
            if not fillers:
                pace['due'] = 0.0

        for idx, (ci, pair, order, se, pv, pv1, rcp_stt) in enumerate(seq):
            njt = len(order)
            if pair == 0:
                if ci + 1 < nci:
                    fillers.extend(proj_units(ci + 1))
                if ci > 0:
                    fillers.extend(outproj_unit(ci - 1, ot, otn_cis[ci - 1])
                                   for ot in range(8))
            # remaining fill slots until the next deadline (chunk boundary)
            slots = 4 * (ci + 1) * (2 - pair)
            wt = len(fillers) / slots if slots else 1.0
            if idx == 0:
                se(order[0])
                se(order[1])
            for k in range(2, njt):
                se(order[k])
                # diag j-tiles have little PE work but a long exp/mask
                # chain: weight fillers toward them
                pace_fill(wt * (2.0 if order[k] >= 4 * ci else 0.8))
                pv(order[k - 2])
            if pair == 1 and ci + 1 < nci:
                # force any not-yet-emitted proj(ci+1) BEFORE the next
                # chunk's scores (they read qk_sb[*][ci+1])
                fill(len(fillers))
            nxt = seq[idx + 1] if idx + 1 < len(seq) else None
            if nxt is not None:
                nxt[3](nxt[2][0])    # next pair's first se
            pv(order[njt - 2])
            if nxt is not None:
                nxt[3](nxt[2][1])    # next pair's second se
            pv1(order[njt - 1], 0)
            rcp_stt(0)           # DVE starts while PE runs the last PV
            pv1(order[njt - 1], 1)
            fill(1)
            rcp_stt(1)
        fillers.extend(outproj_unit(nci - 1, ot, otn_cis[nci - 1], tail=True)
                       for ot in range(8))
        fill(len(fillers))
    nc.compile()
    return nc


def shard_inputs(x, w_qkv, w_out, t=T):
    """Host-side sharding: returns list of 8 in_maps (bf16)."""
    x = np.asarray(x, dtype=np.float32)
    w_qkv = np.asarray(w_qkv, dtype=np.float32)
    w_out = np.asarray(w_out, dtype=np.float32)
    wq = w_qkv[0:D].reshape(H, HD, D)
    wk = w_qkv[D:2 * D].reshape(H, HD, D)
    wv_ = w_qkv[2 * D:3 * D].reshape(H, HD, D)
    in_maps = []
    for core in range(NCORES):
        b, g = core // 4, core % 4
        hs = [4 * g + i for i in range(HPC)]
        xt = np.ascontiguousarray(x[b, :t].T).astype(bfloat16)  # [D, t]
        cols = []
        for pair in range(2):
            hA, hB = hs[2 * pair], hs[2 * pair + 1]
            cols.append(np.concatenate([wq[hA].T, wq[hB].T], axis=1))  # q tile
            cols.append(np.concatenate([wk[hA].T, wk[hB].T], axis=1))  # k tile
        wqk_c = np.ascontiguousarray(np.concatenate(cols, axis=1)).astype(bfloat16)
        wv_c = np.ascontiguousarray(
            np.concatenate([wv_[h].T for h in hs], axis=1)).astype(bfloat16)
        # wo[dd, pair, o] = w_out[o, head(pair, dd//64)*64 + dd%64]
        wo_c = np.ascontiguousarray(np.stack([
            np.concatenate(
                [w_out[:, hs[2 * p] * HD:(hs[2 * p] + 1) * HD].T,
                 w_out[:, hs[2 * p + 1] * HD:(hs[2 * p + 1] + 1) * HD].T],
                axis=0)
            for p in range(2)], axis=1)).astype(bfloat16)           # [128, 2, D]
        in_maps.append({"xt": xt, "wqk": wqk_c, "wv": wv_c, "wo": wo_c})
    return in_maps


def kernel(x, w_qkv, w_out, _trace=False):
    global LAST_RESULTS
    in_maps = shard_inputs(x, w_qkv, w_out)
    nc = build_bass()
    res = run_bass_kernel_spmd(
        nc, in_maps, core_ids=list(range(NCORES)), trace=_trace
    )
    LAST_RESULTS = res
    out = np.zeros((B, T, D), dtype=np.float32)
    for core in range(NCORES):
        b = core // 4
        out[b] += res.results[core]["outp"].astype(np.float32).T
    return out
